# revision 36
# baseline (speedup 1.0000x reference)
"""Distance-aware multi-head attention on 8 trn2 NeuronCores.

Sharding: pure data-parallel over batch (B=8 -> one batch element per core,
no collectives).  Per core, the dominant costs are (a) streaming the
dist_encoding slice (fp8 on the wire: 16.8MB) and (b) the PE
ldweights+matmul stream that contracts it with Wd (1024 stationary tiles).

Math per core (batch b):
  Q^T_h [64,512q]  = (Wq/8)^T x^T          (scale folded into Wq)
  K^T_h [64,512q]  = Wk^T x^T
  V_kt  [128k,512(h,d)] = x W v
  biasT[k,q,h]     = pair-packed dist tiles (stationary) @ blockdiag(Wd,Wd)
  S(h,kt)[128k,256q] = K^T_h(kt)^T Q^T_h  + biasT(strided gather)
  expT = Exp(S + madd_k + bd_h)            (ACT per-partition bias = mask fill)
  AV(h)[65,256q]   = sum_kt [V_h | 1]^T expT   (row 64 = softmax denominator)
  nm[h,q] = mask_q[q] / denom[h,q]; broadcast via row-select matmul
  attnOT[hd,q] = AV[0:64] * nm ;  out[q,:] = attnOT^T Wo (*mask_q via nm)

Stream order is q-half-major / k-half-minor so each q-half's full attention
pipeline runs under the next q-half's dist DMA, shrinking the serial tail.
Weight DMAs ride the ACT HWDGE ring so the dist stream owns the SP ring
from t=0.  A post-pass consolidates per-matmul semaphore increments into
one inc per PSUM-bank group (EVT_SEM writes otherwise serialize ~26ns/MM).
"""

import os
import sys
import threading

for p in ("/opt/trn_rl_repo/concourse", "/opt/trn_rl_repo", "/opt/pypackages"):
    if p not in sys.path:
        sys.path.insert(0, p)

import numpy as np
import ml_dtypes

BF16 = ml_dtypes.bfloat16
FP8 = ml_dtypes.float8_e4m3

B = 8
N = 512          # sequence length
H = 512          # hidden
NH = 8           # heads
D = 64           # head dim
DD = 64          # dist dim
SCALE = float(np.sqrt(D))
NKH = 2          # k halves (256 each)
NQP = N // 2     # 256 q-pairs
NKW = 256        # k within half
NKT = 4          # 128-wide k tiles
NQB = 4          # 128-wide q tiles
QG = 32          # q-pairs per dist DMA chunk
NQG = NQP // QG  # 8 chunks per k-half

DIST_FP8 = bool(int(os.environ.get("KERNEL_DIST_FP8", "1")))
SEM_CONSOLIDATE = bool(int(os.environ.get("KERNEL_SEM_CONS", "0")))
SEM_CONS_SCOPE = os.environ.get("KERNEL_SEM_CONS_SCOPE", "bias")

_lock = threading.Lock()
_cache = {}


def _build_bass(reps=1, mode='full', dist_eng='sync', loop_reps=0,
                dist_fp8=DIST_FP8):
    import concourse.bass as bass
    import concourse.mybir as mybir
    import concourse.tile as tile

    f32 = mybir.dt.float32
    bf16 = mybir.dt.bfloat16
    ddt = mybir.dt.float8e4 if dist_fp8 else bf16
    Exp = mybir.ActivationFunctionType.Exp
    add_op = mybir.AluOpType.add
    mult_op = mybir.AluOpType.mult

    nc = bass.Bass()

    dist_d = nc.dram_tensor("distH", [NKH, 128, NQP * NKW], ddt, kind="ExternalInput")
    bigw_d = [
        nc.dram_tensor(f"bw{i}", [128, 4 * H], bf16, kind="ExternalInput")
        for i in range(5)
    ]
    wdd_d = nc.dram_tensor("wdd", [128, 16], ddt, kind="ExternalInput")
    ident_d = nc.dram_tensor("ident", [128, 128], bf16, kind="ExternalInput")
    madh_d = nc.dram_tensor("madh", [128, NH * NKT], f32, kind="ExternalInput")
    mqrow_d = nc.dram_tensor("mqrow", [1, N], f32, kind="ExternalInput")
    out_d = nc.dram_tensor("out", [N, H], f32, kind="ExternalOutput")

    with tile.TileContext(nc) as tc:
        with (
            tc.tile_pool(name="wpool", bufs=1) as wpool,
            tc.tile_pool(name="dpool", bufs=6) as dpool,
            tc.tile_pool(name="spool", bufs=1) as spool,
            tc.tile_pool(name="ps", bufs=2, space="PSUM") as ps,
        ):
            # ---- weights / constants on the ACT HWDGE ring (SP ring is the
            # dist stream's) ----
            wdd_raw = wpool.tile([128, 16], ddt, tag="wddr", name="wdd_raw")
            nc.scalar.dma_start(wdd_raw[:], wdd_d[:])
            madh_raw = wpool.tile([128, NH * NKT], f32, tag="madhr", name="madh_raw")
            nc.scalar.dma_start(madh_raw[:], madh_d[:])
            bw = []
            for i in range(5):
                t = wpool.tile([128, 4 * H], bf16, tag=f"bw{i}", name=f"bw{i}")
                nc.scalar.dma_start(t[:], bigw_d[i][:])
                bw.append(t)
            mqrow = wpool.tile([1, N], f32, tag="mqrow", name="mqrow_t")
            nc.scalar.dma_start(mqrow[:], mqrow_d[:])
            ident = wpool.tile([128, 128], bf16, tag="ident", name="ident_t")
            nc.scalar.dma_start(ident[:], ident_d[:])

            def wslice(i):
                return [bw[i][:, c * H:(c + 1) * H] for c in range(4)]

            xT, wq, wk, wv, wo = (wslice(i) for i in range(5))

            wdd = wpool.tile([128, 16], ddt, tag="wdd", name="wdd_t")
            nc.vector.tensor_copy(wdd[:], wdd_raw[:])
            madh = wpool.tile([128, NH * NKT], f32, tag="madh", name="madh_t")
            nc.vector.tensor_copy(madh[:], madh_raw[:])
            ones64 = wpool.tile([1, 64], f32, tag="ones64", name="ones64")
            nc.vector.memset(ones64[:], 1.0)
            absorb2 = wpool.tile([1, 64], bf16, tag="absorb2", name="absorb2")

            _loop_cm = tc.For_i(0, loop_reps, 1) if loop_reps else None
            if _loop_cm is not None:
                _loop_cm.__enter__()
            for _rep in range(reps):
              full = (mode == 'full')
              distmm = mode in ('full', 'distmm_c')
              dscale = (1.0 / 64.0) if dist_fp8 else 1.0

              bigT = [
                  spool.tile([128, NQP * 16], bf16, tag=f"bigT{kt}", name=f"bigT{kt}")
                  for kt in range(NKT)
              ]
              vext = [
                  spool.tile([128, NH * (D + 1)], bf16, tag=f"vx{kt}", name=f"vx{kt}")
                  for kt in range(NKT)
              ]
              expT = [[None] * NKT for _ in range(NH)]
              attnOT = [
                  spool.tile([128, N], bf16, tag=f"aot{p}", name=f"aot{p}")
                  for p in range(4)
              ]
              QT = []
              KT = []

              def chunk(kh, qg, all_act=False):
                  dt_ = dpool.tile([128, QG * NKW], ddt, tag="dist", name="dist_t")
                  src = dist_d[kh, :, qg * QG * NKW:(qg + 1) * QG * NKW]
                  getattr(nc, dist_eng).dma_start(dt_[:], src)
                  if not distmm:
                      nc.vector.tensor_copy(absorb2[:], dt_[0:1, 0:64])
                      return
                  for kwh in range(2):
                      kt = 2 * kh + kwh
                      bank = ps.tile([128, QG * 16], f32, tag="psBIAS",
                                     name="psC", bufs=2)
                      for ql in range(QG):
                          lhsT = dt_[:, ql * NKW + kwh * 128:
                                     ql * NKW + kwh * 128 + 128]
                          nc.tensor.matmul(
                              bank[:, ql * 16:(ql + 1) * 16], lhsT, wdd[:],
                              start=True, stop=True,
                          )
                      dst = bigT[kt][:, qg * QG * 16:(qg + 1) * QG * 16]
                      if kwh == 0 or all_act:
                          nc.scalar.mul(dst, bank[:], dscale)
                      else:
                          nc.vector.tensor_scalar_mul(dst, bank[:], dscale)

              def qk_projections():
                  for dst, w in ((QT, wq), (KT, wk)):
                      for hp in range(4):
                          acc = ps.tile([128, N], f32, tag="ps512", name="psA",
                                        bufs=2)
                          for c in range(4):
                              nc.tensor.matmul(
                                  acc[:], w[c][:, hp * 128:(hp + 1) * 128],
                                  xT[c], start=(c == 0), stop=(c == 3),
                              )
                          t = spool.tile([128, N], bf16,
                                         tag=f"qk_{w is wq}_{hp}",
                                         name=f"qk_{w is wq}_{hp}")
                          nc.scalar.copy(t[:], acc[:])
                          dst.append(t)

              def v_projections():
                  for kt in range(NKT):
                      acc = ps.tile([128, H], f32, tag="ps512", name="psV",
                                    bufs=2)
                      for c in range(4):
                          nc.tensor.matmul(
                              acc[:], xT[c][:, kt * 128:(kt + 1) * 128], wv[c],
                              start=(c == 0), stop=(c == 3),
                          )
                      v3 = vext[kt][:].rearrange("p (h c) -> p h c", c=D + 1)
                      nc.gpsimd.memset(v3[:, :, D:D + 1], 1.0)
                      nc.scalar.copy(
                          v3[:, :, 0:D],
                          acc[:].rearrange("p (h c) -> p h c", c=D),
                      )

              _s_state = {"tile": None, "idx": 0}

              def score_exp(h, kt, qh=None, q0=None, qn=None,
                            bias_via_pe=False):
                  if q0 is None:
                      q0, qn = qh * (N // 2), N // 2
                  # two S tiles share one PSUM bank (subrange-tracked) so up
                  # to 6 score units are in flight on 3 banks; full-width
                  # units take a whole bank
                  if qn > N // 2:
                      S = ps.tile([128, N], f32, tag="psS2", name="psS2",
                                  bufs=2)[:, 0:qn]
                      _s_state["idx"] = 0
                      _s_state["tile"] = None
                  else:
                      if _s_state["idx"] % 2 == 0:
                          _s_state["tile"] = ps.tile([128, N], f32,
                                                     tag="psS2", name="psS2",
                                                     bufs=2)
                      half = _s_state["idx"] % 2
                      _s_state["idx"] += 1
                      hw_ = N // 2
                      S = _s_state["tile"][:, half * hw_:half * hw_ + qn]
                  p0 = (h % 2) * 64
                  nc.tensor.matmul(
                      S,
                      KT[h // 2][p0:p0 + 64, kt * 128:(kt + 1) * 128],
                      QT[h // 2][p0:p0 + 64, q0:q0 + qn],
                      start=True, stop=not bias_via_pe,
                  )
                  if bias_via_pe:
                      # tail units: add the bias on the PE (S += I @ bigT
                      # slice) so the serial tail skips the slow strided DVE
                      # add; the PE has slack in the back half
                      nc.tensor.matmul(
                          S, ident[:],
                          bigT[kt][:, 8 * q0 + h:8 * (q0 + qn):8],
                          start=False, stop=True,
                      )
                  else:
                      nc.vector.tensor_tensor(
                          S, S, bigT[kt][:, 8 * q0 + h:8 * (q0 + qn):8],
                          add_op)
                  if expT[h][kt] is None:
                      expT[h][kt] = spool.tile([128, N], bf16,
                                               tag=f"expT{h}_{kt}",
                                               name=f"expT{h}_{kt}")
                  idx = h * NKT + kt
                  nc.scalar.activation(
                      expT[h][kt][:, q0:q0 + qn], S, Exp,
                      bias=madh[:, idx:idx + 1], scale=1.0,
                  )

              def attn_v_stage1(h, q0, qn):
                  """AV accumulation + denominators; nmb broadcast deferred
                  so a stalled nmb matmul never blocks the next unit's AV
                  matmuls in the PE FIFO."""
                  bank = ps.tile([128, N], f32, tag="psAVB", name="psAVB",
                                 bufs=2)
                  AV = bank[0:65, 0:qn]
                  for kt in range(NKT):
                      nc.tensor.matmul(
                          AV, vext[kt][:, h * (D + 1):(h + 1) * (D + 1)],
                          expT[h][kt][:, q0:q0 + qn],
                          start=(kt == 0), stop=(kt == NKT - 1),
                      )
                  rs2 = spool.tile([1, N // 2], f32, tag="rsr", name=f"rsr{h}",
                                   bufs=4)
                  nc.vector.scalar_tensor_tensor(
                      rs2[:, 0:qn], bank[64:65, 0:qn], 1.0,
                      mqrow[:, q0:q0 + qn], mult_op, mult_op)
                  nm = spool.tile([1, N // 2], f32, tag="nm", name=f"nm{h}",
                                  bufs=4)
                  nc.vector.reciprocal_approx_fast(nm[:, 0:qn], rs2[:, 0:qn])
                  return (bank, nm, h, q0, qn)

              def attn_v_stage2(st):
                  bank, nm, h, q0, qn = st
                  nmb = bank[0:64, N // 2:N // 2 + qn]
                  nc.tensor.matmul(nmb, ones64[:], nm[:, 0:qn], start=True,
                                   stop=True)
                  nmb_sb = spool.tile([64, N // 2], f32, tag="nmbsb",
                                      name=f"nmbsb{h}", bufs=4)
                  nc.scalar.copy(nmb_sb[:, 0:qn], nmb)
                  dst = attnOT[h // 2][(h % 2) * 64:(h % 2) * 64 + 64,
                                      q0:q0 + qn]
                  nc.vector.tensor_tensor(dst, bank[0:64, 0:qn],
                                          nmb_sb[:, 0:qn], mult_op)

              def attn_v_block(q0, qn):
                  pend = []
                  for h in range(NH):
                      pend.append(attn_v_stage1(h, q0, qn))
                      if len(pend) > 1:
                          attn_v_stage2(pend.pop(0))
                  for st in pend:
                      attn_v_stage2(st)

              def oproj(qb):
                  O = ps.tile([128, H], f32, tag="ps512", name="psO", bufs=2)
                  for c in range(4):
                      nc.tensor.matmul(
                          O[:], attnOT[c][:, qb * 128:(qb + 1) * 128], wo[c],
                          start=(c == 0), stop=(c == 3),
                      )
                  ot = spool.tile([128, H], f32, tag="osb", name="osb", bufs=2)
                  nc.scalar.copy(ot[:], O[:])
                  nc.scalar.dma_start(out_d[qb * 128:(qb + 1) * 128, :], ot[:])

              KT01FULL = int(os.environ.get("KERNEL_KT01FULL", "0"))
              BIAS_PE_TAIL = bool(int(os.environ.get("KERNEL_BIAS_PE_TAIL",
                                                     "0")))
              # ---- q-half-major / k-half-minor stream + pipeline ----
              for qg in range(NQG // 2):
                  chunk(0, qg)
              for qg in range(NQG // 2):
                  chunk(1, qg)
              if full:
                  qk_projections()
                  if not KT01FULL:
                      # kt-major so consecutive score MMs alternate between
                      # array rows 0-63 (even heads) and 64-127 (odd heads):
                      # row-disjoint LDWEIGHTS overlap in-flight matmuls
                      for kt in (0, 1):
                          for h in range(NH):
                              score_exp(h, kt, 0)
                  v_projections()
                  for kt in (2, 3):
                      for h in range(NH):
                          score_exp(h, kt, 0)
              for qg in range(NQG // 2, NQG):
                  chunk(0, qg)
              if full:
                  if KT01FULL:
                      # single full-q score/exp per (h, kt01): halves per-op
                      # overhead; bias cols for all q are ready once the
                      # kh0 stream completes
                      for h in range(NH):
                          for kt in (0, 1):
                              score_exp(h, kt, q0=0, qn=N)
                      attn_v_block(0, N // 2)
                  else:
                      attn_v_block(0, N // 2)
                      for kt in (0, 1):
                          for h in range(NH):
                              score_exp(h, kt, 1)
              # final k-half: stream + process per q-block so the last
              # q-block's score/AV/oproj chains overlap the stream
              QSPLIT = int(os.environ.get("KERNEL_QSPLIT", "0"))
              for qg in (NQG // 2, NQG // 2 + 1):
                  chunk(1, qg, all_act=True)
              if full:
                  for qb in (0, 1):
                      oproj(qb)
              if full and QSPLIT:
                  # q-block kt23 scores: only need the two chunks just issued
                  for h in range(NH):
                      for kt in (2, 3):
                          score_exp(h, kt, q0=N // 2, qn=N // 4)
              for qg in (NQG // 2 + 2, NQG // 2 + 3):
                  chunk(1, qg, all_act=True)
              if full and not QSPLIT:
                  for kt in (2, 3):
                      for h in range(NH):
                          score_exp(h, kt, 1, bias_via_pe=BIAS_PE_TAIL)
                  attn_v_block(N // 2, N // 2)
                  oproj(2)
                  oproj(3)
              if full and QSPLIT:
                  attn_v_block(N // 2, N // 4)
                  for h in range(NH):
                      for kt in (2, 3):
                          score_exp(h, kt, q0=3 * N // 4, qn=N // 4)
                  oproj(2)
                  attn_v_block(3 * N // 4, N // 4)
                  oproj(3)
            if _loop_cm is not None:
                _loop_cm.__exit__(None, None, None)

    _strip_self_waits(nc)
    if SEM_CONSOLIDATE:
        _consolidate_sem_incs(nc)
    _fit_sync_limits(nc)
    from concourse.library_overlay import lower_extended_insts
    lower_extended_insts(nc)
    return nc


def _strip_self_waits(nc):
    """Remove same-engine semaphore waits (vacuous: engines execute in
    program order) so instructions fit walrus' per-instruction sync-command
    limits."""
    import concourse.mybir as mybir
    eng_sem = {
        mybir.EngineType.PE: "PE_",
        mybir.EngineType.DVE: "DVE_",
        mybir.EngineType.Activation: "Activation_",
        mybir.EngineType.SP: "SP_",
        mybir.EngineType.Pool: "Pool_",
    }
    for blk in nc.m.functions[0].blocks:
        for i in blk.instructions:
            si = i.sync_info
            if not si or not si.on_wait:
                continue
            eng = getattr(i, "engine", None)
            pref = eng_sem.get(eng)
            if pref is not None:
                kept = [w for w in si.on_wait if not w.ant_name.startswith(pref)]
                if len(kept) != len(si.on_wait):
                    si.on_wait = kept
            # dist-stream DMAs: a PE wait (WAR vs this slot's readers)
            # transitively implies the predecessor DMA completed, making a
            # coexisting cross-lane DMAHW wait redundant.
            if type(i).__name__ == "InstDMACopy" and any(
                "dist_t" in getattr(o, "memref", "") for o in i.outs
            ):
                w = si.on_wait
                if len(w) > 1 and any(x.ant_name.startswith("PE_") for x in w):
                    si.on_wait = [
                        x for x in w if not x.ant_name.startswith("DMAHW")
                    ]


def _consolidate_sem_incs(nc):
    """Drop the per-matmul `sem-inc +1` on all but the last matmul of each
    PSUM-bank group, remapping every wait threshold on that semaphore to the
    new (smaller) cumulative counts.  PE executes its queue in order, so a
    consumer that waited for "32 matmuls done" can equivalently wait for
    "bank-group #1 done"; the EVT_SEM write per matmul is pure overhead.

    Runs break at: a waiting instruction (its wait may transitively depend
    on an earlier group's inc), a different target semaphore, a multi-update
    or non-unit-value update, any non-MM/LDW instruction, and any change of
    output tensor (keeps all existing wait thresholds exactly on run
    boundaries).  Waits are remapped as v -> index of the run containing
    the v'th original inc; mid-run thresholds round up to the run's end,
    which is correct (conservative) because nothing inside a run is waited
    on by anything a run member depends on (first-members carry the run's
    only waits)."""
    CONS_TYPES = {"InstMatmult", "InstLdweights"}

    for fn in nc.m.functions:
        # sems are reset between For_i iterations; each block's waits refer
        # to counts from its own block.  Refuse any sem whose updates span
        # blocks (the wait remap below assumes one producing block).
        sem_block_count = {}
        for blk in fn.blocks:
            seen = set()
            for i in blk.instructions:
                si = i.sync_info
                if not si:
                    continue
                for u in si.on_update:
                    seen.add(u.ant_name)
            for s in seen:
                sem_block_count[s] = sem_block_count.get(s, 0) + 1
        for blk in fn.blocks:
            pe_insts = [
                i for i in blk.instructions
                if getattr(i, "engine", None) is not None
                and getattr(i.engine, "name", "") == "PE"
            ]
            # collect candidate sems updated by PE in this block
            sem_updates = {}
            for i in pe_insts:
                si = i.sync_info
                if not si:
                    continue
                for u in si.on_update:
                    sem_updates.setdefault(u.ant_name, []).append((i, u))
            for sem, upds in sem_updates.items():
                # eligibility: every update on this sem is a lone sem-inc +1
                # on a MM/LDW instruction
                ok = all(
                    type(i).__name__ in CONS_TYPES
                    and len(i.sync_info.on_update) == 1
                    and u.update_mode == "sem-inc"
                    and u.update_value == 1
                    for i, u in upds
                )
                if not ok or len(upds) < 4 or sem_block_count.get(sem, 0) > 1:
                    continue
                # waits on this sem anywhere must be plain immediate geq
                waits = []
                for b2 in fn.blocks:
                    for i in b2.instructions:
                        si = i.sync_info
                        if not si:
                            continue
                        for w in si.on_wait:
                            if w.ant_name == sem:
                                waits.append(w)
                if any(
                    getattr(w, "wait_reg", None) is not None
                    or w.wait_value is None
                    for w in waits
                ):
                    continue
                # form runs over this block's PE instruction order
                run_end_old = []       # old cumulative count at each run end
                survivors = []         # the update-bearing inst ending each run
                cum = 0
                run_len = 0
                run_out = None
                run_last = None

                def flush():
                    nonlocal run_len, run_out, run_last
                    if run_len:
                        run_end_old.append(cum)
                        survivors.append(run_last)
                    run_len = 0
                    run_out = None
                    run_last = None

                for i in pe_insts:
                    si = i.sync_info
                    has_wait = bool(si and si.on_wait)
                    my_upds = [u for u in (si.on_update if si else [])
                               if u.ant_name == sem]
                    if type(i).__name__ not in CONS_TYPES:
                        flush()
                        continue
                    if has_wait and run_len:
                        flush()
                    if not my_upds:
                        if si and si.on_update:
                            flush()   # updates some other sem: break run
                        continue
                    out0 = getattr(i.outs[0], "memref", "") if i.outs else ""
                    if run_len and out0 != run_out:
                        flush()
                    cum += 1
                    run_len += 1
                    run_out = out0
                    run_last = i
                    if SEM_CONS_SCOPE == "bias" and "psC" not in out0:
                        flush()   # consolidate only dist-bias bank groups
                flush()
                if len(survivors) >= len(upds):
                    continue
                # rewrite updates: only survivors keep their +1
                surv_set = {id(i) for i in survivors}
                for i, u in upds:
                    if id(i) not in surv_set:
                        i.sync_info.on_update = [
                            x for x in i.sync_info.on_update
                            if x.ant_name != sem
                        ]
                # remap waits: old threshold v -> first run index r (1-based)
                # with run_end_old[r-1] >= v
                import bisect
                for w in waits:
                    v = w.wait_value
                    if v <= 0:
                        continue
                    r = bisect.bisect_left(run_end_old, v)
                    w.wait_value = min(r + 1, len(run_end_old))


_FITTABLE = {
    "InstMatmult", "InstLdweights", "InstActivation", "InstTensorTensor",
    "InstTensorCopy", "InstTensorScalarPtr", "InstCustomDveAnt",
    "InstMemset", "InstReciprocal", "InstDMACopy", "InstTensorReduce",
    "InstDrain", "InstNoOp", "InstEventSemaphore",
}


def _fit_sync_limits(nc):
    """Walrus' 64B instruction encodings fit 3 sync slots; a wait costs 2,
    an update 1 — so at most ONE wait per instruction.  Hoist excess waits
    onto same-engine NOPs injected just before the instruction — the NX
    sequencer executes the NOP's waits first, which is semantically
    identical."""
    import concourse.mybir as mybir

    for blk in nc.m.functions[0].blocks:
        il = blk.instructions
        out = []
        for inst in il:
            si = inst.sync_info
            if (
                type(inst).__name__ not in _FITTABLE
                or si is None
                or not si.on_wait
            ):
                out.append(inst)
                continue
            waits = list(si.on_wait)
            if len(waits) <= 1:
                out.append(inst)
                continue
            excess, kept = waits[:-1], waits[-1:]
            for j, w in enumerate(excess):
                nop = mybir.InstNoOp(
                    name=f"{inst.name}-hw{j}",
                    engine=inst.engine,
                    ins=[],
                    outs=[],
                    sync_info=mybir.SyncInfo(on_wait=[w], on_update=[]),
                )
                out.append(nop)
            si.on_wait = kept
            out.append(inst)
        il[:] = out


def _get_bass():
    with _lock:
        key = ("nc", DIST_FP8)
        if key not in _cache:
            _cache[key] = _build_bass(dist_fp8=DIST_FP8)
        return _cache[key]


def _prep_core(b, x, dist, mask, bd, ddtype):
    """Build the per-core input map for batch element b."""
    xT = np.ascontiguousarray(x[b].T).astype(BF16)
    d = dist[b].reshape(NQP, 2, NKH, NKW, DD)
    distH = np.ascontiguousarray(d.transpose(2, 1, 4, 0, 3)).reshape(
        NKH, 128, NQP * NKW
    ).astype(ddtype)
    mk = mask[b].astype(np.float32)
    madd = np.where(mk > 0.5, 0.0, -1e9).astype(np.float32)
    madh = np.empty((128, NH * NKT), np.float32)
    for h in range(NH):
        for kt in range(NKT):
            madh[:, h * NKT + kt] = madd[kt * 128:(kt + 1) * 128] + float(bd[h])
    return {
        "distH": distH,
        "xT": xT,
        "madh": madh,
        "mqrow": np.where(mk > 0.5, 1.0, 1e30).astype(
            np.float32).reshape(1, N),
    }


def _cpu_reference(x, dist, mask, Wq, bq, Wk, bk, Wv, bv, Wo, bo, Wd, bd):
    """NumPy fallback for input shapes/bias values the Bass kernel doesn't
    hardcode.  Never taken for the reference setup_inputs()."""
    Bn, Nn, Hn = x.shape
    nh = Wd.shape[1]
    dh = Hn // nh
    sc = float(np.sqrt(dh))

    def heads(t):
        return t.reshape(Bn, Nn, nh, dh).transpose(0, 2, 1, 3)

    q = heads(x @ Wq + bq)
    k = heads(x @ Wk + bk)
    v = heads(x @ Wv + bv)
    scores = np.einsum("bhqd,bhkd->bhqk", q, k) / sc
    scores = scores + (dist @ Wd + bd).transpose(0, 3, 1, 2)
    scores = np.where(mask[:, None, None, :], scores, -1e9)
    scores = scores - scores.max(axis=-1, keepdims=True)
    e = np.exp(scores)
    attn = e / e.sum(axis=-1, keepdims=True)
    attn = attn * mask[:, None, :, None].astype(attn.dtype)
    out = np.einsum("bhqk,bhkd->bhqd", attn, v)
    out = out.transpose(0, 2, 1, 3).reshape(Bn, Nn, Hn)
    out = (out @ Wo + bo) * mask[:, :, None].astype(out.dtype)
    return out.astype(np.float32)


def kernel(x, dist_encoding, mask, Wq, bq, Wk, bk, Wv, bv, Wo, bo, Wd, bd,
           trace=False):
    from concourse.bass_utils import run_bass_kernel_spmd

    x = np.asarray(x, dtype=np.float32)
    dist = np.asarray(dist_encoding, dtype=np.float32)
    mask = np.asarray(mask)
    Wq = np.asarray(Wq, np.float32); Wk = np.asarray(Wk, np.float32)
    Wv = np.asarray(Wv, np.float32); Wo = np.asarray(Wo, np.float32)
    Wd = np.asarray(Wd, np.float32)
    bq = np.asarray(bq, np.float32); bk = np.asarray(bk, np.float32)
    bv = np.asarray(bv, np.float32); bo = np.asarray(bo, np.float32)
    bd = np.asarray(bd, np.float32)
    if (np.any(bq) or np.any(bk) or np.any(bv) or np.any(bo)
            or x.shape != (B, N, H) or dist.shape != (B, N, N, DD)):
        return _cpu_reference(x, dist, mask, Wq, bq, Wk, bk, Wv, bv,
                              Wo, bo, Wd, bd)

    # shared (replicated) weights
    wq_s = np.ascontiguousarray(Wq / SCALE).astype(BF16)
    wk_b = np.ascontiguousarray(Wk).astype(BF16)
    wv_b = np.ascontiguousarray(Wv).astype(BF16)
    wo_b = np.ascontiguousarray(Wo).astype(BF16)
    wdd = np.zeros((128, 16), np.float32)
    wdd[0:64, 0:8] = Wd
    wdd[64:128, 8:16] = Wd
    if DIST_FP8:
        wdd = (wdd * 64.0).astype(FP8)
    else:
        wdd = wdd.astype(BF16)

    from concurrent.futures import ThreadPoolExecutor
    ddtype = FP8 if DIST_FP8 else BF16
    with ThreadPoolExecutor(max_workers=8) as ex:
        percore = list(ex.map(
            lambda b: _prep_core(b, x, dist, mask, bd, ddtype),
            range(B),
        ))
    in_maps = []
    for b in range(B):
        m = dict(percore[b])
        xT_b = m.pop("xT")
        for i, w in enumerate((xT_b, wq_s, wk_b, wv_b, wo_b)):
            m[f"bw{i}"] = np.ascontiguousarray(
                w.reshape(4, 128, H).transpose(1, 0, 2).reshape(128, 4 * H))
        m["wdd"] = wdd
        m["ident"] = np.eye(128, dtype=BF16)
        in_maps.append(m)

    nc = _get_bass()
    kernel.last_in_maps = in_maps
    res = run_bass_kernel_spmd(nc, in_maps, list(range(B)), trace=False)
    out = np.stack([res.results[b]["out"] for b in range(B)]).astype(np.float32)
    if trace:
        kernel.last_exec_time_ns = res.exec_time_ns
        kernel.last_results = res
    return out


def bench_exec_ns(in_maps=None, iters=16, reps2=129, mode='full', dist_eng='sync'):
    """Estimate per-execution HW time: steady-state wall time of the jitted
    SPMD kernel with device-resident inputs, minus bare dispatch overhead."""
    import time
    import jax
    import jax.numpy as jnp
    from jax.sharding import Mesh, PartitionSpec
    from jax.experimental.shard_map import shard_map
    import concourse.bass2jax as b2j
    import concourse.mybir as mybir

    if in_maps is None:
        in_maps = kernel.last_in_maps
    n_cores = len(in_maps)

    nc = _build_bass(mode=mode, dist_eng=dist_eng, loop_reps=1,
                     dist_fp8=DIST_FP8)
    ncR = _build_bass(mode=mode, dist_eng=dist_eng, loop_reps=reps2,
                      dist_fp8=DIST_FP8)
    partition_name = nc.partition_id_tensor.name if nc.partition_id_tensor else None
    in_names, out_names, out_avals, zero_outs = [], [], [], []
    for alloc in nc.m.functions[0].allocations:
        if not isinstance(alloc, mybir.MemoryLocationSet):
            continue
        name = alloc.memorylocations[0].name
        if alloc.kind == "ExternalInput":
            if name != partition_name:
                in_names.append(name)
        elif alloc.kind == "ExternalOutput":
            out_names.append(name)
            shape = tuple(alloc.tensor_shape)
            dtype = mybir.dt.np(alloc.dtype)
            out_avals.append(jax.core.ShapedArray(shape, dtype))
            zero_outs.append(np.zeros(shape, dtype))
    n_params = len(in_names)
    n_outs = len(out_avals)
    all_in_names = list(in_names) + out_names
    if partition_name is not None:
        all_in_names.append(partition_name)

    def _mk_body(nc_):
        def _body(*args):
            operands = list(args)
            if partition_name is not None:
                operands.append(b2j.partition_id_tensor())
            outs = b2j._bass_exec_p.bind(
                *operands,
                out_avals=tuple(out_avals),
                in_names=tuple(all_in_names),
                out_names=tuple(out_names),
                lowering_input_output_aliases=(),
                sim_require_finite=True,
                sim_require_nnan=True,
                nc=nc_,
            )
            return tuple(outs)
        return _body

    devices = jax.devices()[:n_cores]
    mesh = Mesh(np.asarray(devices), ("core",))
    in_specs = (PartitionSpec("core"),) * (n_params + n_outs)
    out_specs = (PartitionSpec("core"),) * n_outs
    def make_fn(nc_):
        return jax.jit(
            shard_map(_mk_body(nc_), mesh=mesh,
                      in_specs=in_specs, out_specs=out_specs, check_rep=False),
            keep_unused=True,
        )

    fn = make_fn(nc)
    from jax.sharding import NamedSharding
    shardng = NamedSharding(mesh, PartitionSpec("core"))
    concat_in = [
        jax.device_put(
            np.concatenate([np.asarray(in_maps[c][in_names[i]])
                            for c in range(n_cores)], axis=0), shardng)
        for i in range(n_params)
    ]
    concat_zeros = [
        jax.device_put(
            np.zeros((n_cores * z.shape[0], *z.shape[1:]), z.dtype), shardng)
        for z in zero_outs
    ]
    fnK = make_fn(ncR)

    args = concat_in + concat_zeros
    jax.block_until_ready(fn(*args))
    jax.block_until_ready(fnK(*args))
    t1s, tKs = [], []
    for _ in range(iters):
        t0 = time.perf_counter()
        jax.block_until_ready(fn(*args))
        t1s.append(time.perf_counter() - t0)
        t0 = time.perf_counter()
        jax.block_until_ready(fnK(*args))
        tKs.append(time.perf_counter() - t0)
    t1s.sort(); tKs.sort()
    k = max(3, iters // 3)
    t1 = sum(t1s[:k]) / k
    tK = sum(tKs[:k]) / k
    per = (tK - t1) / (reps2 - 1)
    return {
        "kernel_wall_ns": t1 * 1e9,
        "kernel_wallK_ns": tK * 1e9,
        "exec_est_ns": per * 1e9,
    }


# revision 39
# speedup vs baseline: 1.0546x; 1.0546x over previous
"""Distance-aware multi-head attention on 8 trn2 NeuronCores.

Sharding: pure data-parallel over batch (B=8 -> one batch element per core,
no collectives).  Per core, the dominant costs are (a) streaming the
dist_encoding slice (fp8 on the wire: 16.8MB) and (b) the PE
ldweights+matmul stream that contracts it with Wd (1024 stationary tiles).

Math per core (batch b):
  Q^T_h [64,512q]  = (Wq/8)^T x^T          (scale folded into Wq)
  K^T_h [64,512q]  = Wk^T x^T
  V_kt  [128k,512(h,d)] = x W v
  biasT[k,q,h]     = pair-packed dist tiles (stationary) @ blockdiag(Wd,Wd)
  S(h,kt)[128k,256q] = K^T_h(kt)^T Q^T_h  + biasT(strided gather)
  expT = Exp(S + madd_k + bd_h)            (ACT per-partition bias = mask fill)
  AV(h)[65,256q]   = sum_kt [V_h | 1]^T expT   (row 64 = softmax denominator)
  nm[h,q] = mask_q[q] / denom[h,q]; broadcast via row-select matmul
  attnOT[hd,q] = AV[0:64] * nm ;  out[q,:] = attnOT^T Wo (*mask_q via nm)

Stream order is q-half-major / k-half-minor so each q-half's full attention
pipeline runs under the next q-half's dist DMA, shrinking the serial tail.
Weight DMAs ride the ACT HWDGE ring so the dist stream owns the SP ring
from t=0.  A post-pass consolidates per-matmul semaphore increments into
one inc per PSUM-bank group (EVT_SEM writes otherwise serialize ~26ns/MM).
"""

import os
import sys
import threading

for p in ("/opt/trn_rl_repo/concourse", "/opt/trn_rl_repo", "/opt/pypackages"):
    if p not in sys.path:
        sys.path.insert(0, p)

import numpy as np
import ml_dtypes

BF16 = ml_dtypes.bfloat16
FP8 = ml_dtypes.float8_e4m3

B = 8
N = 512          # sequence length
H = 512          # hidden
NH = 8           # heads
D = 64           # head dim
DD = 64          # dist dim
SCALE = float(np.sqrt(D))
NKH = 2          # k halves (256 each)
NQP = N // 2     # 256 q-pairs
NKW = 256        # k within half
NKT = 4          # 128-wide k tiles
NQB = 4          # 128-wide q tiles
QG = 32          # q-pairs per dist DMA chunk
NQG = NQP // QG  # 8 chunks per k-half

DIST_FP8 = bool(int(os.environ.get("KERNEL_DIST_FP8", "1")))
SEM_CONSOLIDATE = bool(int(os.environ.get("KERNEL_SEM_CONS", "0")))
SEM_CONS_SCOPE = os.environ.get("KERNEL_SEM_CONS_SCOPE", "bias")

_lock = threading.Lock()
_cache = {}


def _build_bass(reps=1, mode='full', dist_eng='sync', loop_reps=0,
                dist_fp8=DIST_FP8):
    import concourse.bass as bass
    import concourse.mybir as mybir
    import concourse.tile as tile

    f32 = mybir.dt.float32
    bf16 = mybir.dt.bfloat16
    ddt = mybir.dt.float8e4 if dist_fp8 else bf16
    Exp = mybir.ActivationFunctionType.Exp
    add_op = mybir.AluOpType.add
    mult_op = mybir.AluOpType.mult

    nc = bass.Bass()

    dist_d = nc.dram_tensor("distH", [NKH, 128, NQP * NKW], ddt, kind="ExternalInput")
    bigw_d = [
        nc.dram_tensor(f"bw{i}", [128, 4 * H], bf16, kind="ExternalInput")
        for i in range(5)
    ]
    wdd_d = nc.dram_tensor("wdd", [128, 16], ddt, kind="ExternalInput")
    ident_d = nc.dram_tensor("ident", [128, 128], bf16, kind="ExternalInput")
    madh_d = nc.dram_tensor("madh", [128, NH * NKT], f32, kind="ExternalInput")
    mqrow_d = nc.dram_tensor("mqrow", [1, N], f32, kind="ExternalInput")
    out_d = nc.dram_tensor("out", [N, H], f32, kind="ExternalOutput")

    with tile.TileContext(nc) as tc:
        with (
            tc.tile_pool(name="wpool", bufs=1) as wpool,
            tc.tile_pool(name="dpool", bufs=8) as dpool,
            tc.tile_pool(name="spool", bufs=1) as spool,
            tc.tile_pool(name="ps", bufs=2, space="PSUM") as ps,
        ):
            # ---- weights / constants on the ACT HWDGE ring (SP ring is the
            # dist stream's) ----
            wdd_raw = wpool.tile([128, 16], ddt, tag="wddr", name="wdd_raw")
            nc.scalar.dma_start(wdd_raw[:], wdd_d[:])
            madh_raw = wpool.tile([128, NH * NKT], f32, tag="madhr", name="madh_raw")
            nc.scalar.dma_start(madh_raw[:], madh_d[:])
            bw = []
            for i in range(5):
                t = wpool.tile([128, 4 * H], bf16, tag=f"bw{i}", name=f"bw{i}")
                nc.scalar.dma_start(t[:], bigw_d[i][:])
                bw.append(t)
            mqrow = wpool.tile([1, N], f32, tag="mqrow", name="mqrow_t")
            nc.scalar.dma_start(mqrow[:], mqrow_d[:])
            ident = wpool.tile([128, 128], bf16, tag="ident", name="ident_t")
            nc.scalar.dma_start(ident[:], ident_d[:])

            def wslice(i):
                return [bw[i][:, c * H:(c + 1) * H] for c in range(4)]

            xT, wq, wk, wv, wo = (wslice(i) for i in range(5))

            wdd = wpool.tile([128, 16], ddt, tag="wdd", name="wdd_t")
            nc.vector.tensor_copy(wdd[:], wdd_raw[:])
            madh = wpool.tile([128, NH * NKT], f32, tag="madh", name="madh_t")
            nc.vector.tensor_copy(madh[:], madh_raw[:])
            ones64 = wpool.tile([1, 64], f32, tag="ones64", name="ones64")
            nc.vector.memset(ones64[:], 1.0)
            absorb2 = wpool.tile([1, 64], bf16, tag="absorb2", name="absorb2")

            _loop_cm = tc.For_i(0, loop_reps, 1) if loop_reps else None
            if _loop_cm is not None:
                _loop_cm.__enter__()
            for _rep in range(reps):
              full = (mode == 'full')
              distmm = mode in ('full', 'distmm_c')
              dscale = (1.0 / 64.0) if dist_fp8 else 1.0

              bigT = [
                  spool.tile([128, NQP * 16], bf16, tag=f"bigT{kt}", name=f"bigT{kt}")
                  for kt in range(NKT)
              ]
              vext = [
                  spool.tile([128, NH * (D + 1)], bf16, tag=f"vx{kt}", name=f"vx{kt}")
                  for kt in range(NKT)
              ]
              expT = [[None] * NKT for _ in range(NH)]
              attnOT = [
                  spool.tile([128, N], bf16, tag=f"aot{p}", name=f"aot{p}")
                  for p in range(4)
              ]
              QT = []
              KT = []

              def chunk(kh, qg, all_act=False):
                  dt_ = dpool.tile([128, QG * NKW], ddt, tag="dist", name="dist_t")
                  src = dist_d[kh, :, qg * QG * NKW:(qg + 1) * QG * NKW]
                  getattr(nc, dist_eng).dma_start(dt_[:], src)
                  if not distmm:
                      nc.vector.tensor_copy(absorb2[:], dt_[0:1, 0:64])
                      return
                  for kwh in range(2):
                      kt = 2 * kh + kwh
                      bank = ps.tile([128, QG * 16], f32, tag="ps512",
                                     name="psC", bufs=2)
                      for ql in range(QG):
                          lhsT = dt_[:, ql * NKW + kwh * 128:
                                     ql * NKW + kwh * 128 + 128]
                          nc.tensor.matmul(
                              bank[:, ql * 16:(ql + 1) * 16], lhsT, wdd[:],
                              start=True, stop=True,
                          )
                      dst = bigT[kt][:, qg * QG * 16:(qg + 1) * QG * 16]
                      # all bias-bank muls on ACT: DVE's FIFO paces the
                      # score-chain adds and must stay clear of bulk copies
                      nc.scalar.mul(dst, bank[:], dscale)

              def qk_projections():
                  for dst, w in ((QT, wq), (KT, wk)):
                      for hp in range(4):
                          acc = ps.tile([128, N], f32, tag="ps512", name="psA",
                                        bufs=2)
                          for c in range(4):
                              nc.tensor.matmul(
                                  acc[:], w[c][:, hp * 128:(hp + 1) * 128],
                                  xT[c], start=(c == 0), stop=(c == 3),
                              )
                          t = spool.tile([128, N], bf16,
                                         tag=f"qk_{w is wq}_{hp}",
                                         name=f"qk_{w is wq}_{hp}")
                          nc.scalar.copy(t[:], acc[:])
                          dst.append(t)

              def v_projections():
                  for kt in range(NKT):
                      acc = ps.tile([128, H], f32, tag="ps512", name="psV",
                                    bufs=2)
                      for c in range(4):
                          nc.tensor.matmul(
                              acc[:], xT[c][:, kt * 128:(kt + 1) * 128], wv[c],
                              start=(c == 0), stop=(c == 3),
                          )
                      v3 = vext[kt][:].rearrange("p (h c) -> p h c", c=D + 1)
                      nc.gpsimd.memset(v3[:, :, D:D + 1], 1.0)
                      nc.scalar.copy(
                          v3[:, :, 0:D],
                          acc[:].rearrange("p (h c) -> p h c", c=D),
                      )

              _s_state = {"tile": None, "idx": 0}

              def score_exp(h, kt, qh=None, q0=None, qn=None,
                            bias_via_pe=False):
                  if q0 is None:
                      q0, qn = qh * (N // 2), N // 2
                  # two S tiles share one PSUM bank (subrange-tracked) so up
                  # to 6 score units are in flight on 3 banks; full-width
                  # units take a whole bank
                  if qn > N // 2:
                      S = ps.tile([128, N], f32, tag="psS2", name="psS2",
                                  bufs=3)[:, 0:qn]
                      _s_state["idx"] = 0
                      _s_state["tile"] = None
                  else:
                      if _s_state["idx"] % 2 == 0:
                          _s_state["tile"] = ps.tile([128, N], f32,
                                                     tag="psS2", name="psS2",
                                                     bufs=3)
                      half = _s_state["idx"] % 2
                      _s_state["idx"] += 1
                      hw_ = N // 2
                      S = _s_state["tile"][:, half * hw_:half * hw_ + qn]
                  p0 = (h % 2) * 64
                  nc.tensor.matmul(
                      S,
                      KT[h // 2][p0:p0 + 64, kt * 128:(kt + 1) * 128],
                      QT[h // 2][p0:p0 + 64, q0:q0 + qn],
                      start=True, stop=not bias_via_pe,
                  )
                  if bias_via_pe:
                      # tail units: add the bias on the PE (S += I @ bigT
                      # slice) so the serial tail skips the slow strided DVE
                      # add; the PE has slack in the back half
                      nc.tensor.matmul(
                          S, ident[:],
                          bigT[kt][:, 8 * q0 + h:8 * (q0 + qn):8],
                          start=False, stop=True,
                      )
                  else:
                      nc.vector.tensor_tensor(
                          S, S, bigT[kt][:, 8 * q0 + h:8 * (q0 + qn):8],
                          add_op)
                  if expT[h][kt] is None:
                      expT[h][kt] = spool.tile([128, N], bf16,
                                               tag=f"expT{h}_{kt}",
                                               name=f"expT{h}_{kt}")
                  idx = h * NKT + kt
                  nc.scalar.activation(
                      expT[h][kt][:, q0:q0 + qn], S, Exp,
                      bias=madh[:, idx:idx + 1], scale=1.0,
                  )

              def attn_v_stage1(h, q0, qn):
                  """AV accumulation + denominators; nmb broadcast deferred
                  so a stalled nmb matmul never blocks the next unit's AV
                  matmuls in the PE FIFO."""
                  bank = ps.tile([128, N], f32, tag="psAVB", name="psAVB",
                                 bufs=3)
                  AV = bank[0:65, 0:qn]
                  for kt in range(NKT):
                      nc.tensor.matmul(
                          AV, vext[kt][:, h * (D + 1):(h + 1) * (D + 1)],
                          expT[h][kt][:, q0:q0 + qn],
                          start=(kt == 0), stop=(kt == NKT - 1),
                      )
                  rs2 = spool.tile([1, N // 2], f32, tag="rsr", name=f"rsr{h}",
                                   bufs=4)
                  nc.vector.scalar_tensor_tensor(
                      rs2[:, 0:qn], bank[64:65, 0:qn], 1.0,
                      mqrow[:, q0:q0 + qn], mult_op, mult_op)
                  nm = spool.tile([1, N // 2], f32, tag="nm", name=f"nm{h}",
                                  bufs=4)
                  nc.vector.reciprocal_approx_fast(nm[:, 0:qn], rs2[:, 0:qn])
                  return (bank, nm, h, q0, qn)

              def attn_v_stage2(st):
                  bank, nm, h, q0, qn = st
                  nmb = bank[0:64, N // 2:N // 2 + qn]
                  nc.tensor.matmul(nmb, ones64[:], nm[:, 0:qn], start=True,
                                   stop=True)
                  nmb_sb = spool.tile([64, N // 2], f32, tag="nmbsb",
                                      name=f"nmbsb{h}", bufs=4)
                  nc.scalar.copy(nmb_sb[:, 0:qn], nmb)
                  dst = attnOT[h // 2][(h % 2) * 64:(h % 2) * 64 + 64,
                                      q0:q0 + qn]
                  nc.vector.tensor_tensor(dst, bank[0:64, 0:qn],
                                          nmb_sb[:, 0:qn], mult_op)

              def attn_v_block(q0, qn):
                  pend = []
                  for h in range(NH):
                      pend.append(attn_v_stage1(h, q0, qn))
                      if len(pend) > 1:
                          attn_v_stage2(pend.pop(0))
                  for st in pend:
                      attn_v_stage2(st)

              def oproj(qb):
                  O = ps.tile([128, H], f32, tag="ps512", name="psO", bufs=2)
                  for c in range(4):
                      nc.tensor.matmul(
                          O[:], attnOT[c][:, qb * 128:(qb + 1) * 128], wo[c],
                          start=(c == 0), stop=(c == 3),
                      )
                  ot = spool.tile([128, H], f32, tag="osb", name="osb", bufs=2)
                  nc.scalar.copy(ot[:], O[:])
                  nc.scalar.dma_start(out_d[qb * 128:(qb + 1) * 128, :], ot[:])

              KT01FULL = int(os.environ.get("KERNEL_KT01FULL", "0"))
              BIAS_PE_TAIL = bool(int(os.environ.get("KERNEL_BIAS_PE_TAIL",
                                                     "0")))
              # ---- q-half-major / k-half-minor stream + pipeline ----
              for qg in range(NQG // 2):
                  chunk(0, qg)
              for qg in range(NQG // 2):
                  chunk(1, qg)
              if full:
                  qk_projections()
                  if not KT01FULL:
                      # kt-major so consecutive score MMs alternate between
                      # array rows 0-63 (even heads) and 64-127 (odd heads):
                      # row-disjoint LDWEIGHTS overlap in-flight matmuls
                      for kt in (0, 1):
                          for h in range(NH):
                              score_exp(h, kt, 0)
                  v_projections()
                  for kt in (2, 3):
                      for h in range(NH):
                          score_exp(h, kt, 0)
              for qg in range(NQG // 2, NQG):
                  chunk(0, qg)
              if full:
                  if KT01FULL:
                      # single full-q score/exp per (h, kt01): halves per-op
                      # overhead; bias cols for all q are ready once the
                      # kh0 stream completes
                      for h in range(NH):
                          for kt in (0, 1):
                              score_exp(h, kt, q0=0, qn=N)
                      attn_v_block(0, N // 2)
                  else:
                      attn_v_block(0, N // 2)
                      for kt in (0, 1):
                          for h in range(NH):
                              score_exp(h, kt, 1)
              # final k-half: stream + process per q-block so the last
              # q-block's score/AV/oproj chains overlap the stream
              QSPLIT = int(os.environ.get("KERNEL_QSPLIT", "0"))
              for qg in (NQG // 2, NQG // 2 + 1):
                  chunk(1, qg, all_act=True)
              if full:
                  for qb in (0, 1):
                      oproj(qb)
              if full and QSPLIT:
                  # q-block kt23 scores: only need the two chunks just issued
                  for h in range(NH):
                      for kt in (2, 3):
                          score_exp(h, kt, q0=N // 2, qn=N // 4)
              for qg in (NQG // 2 + 2, NQG // 2 + 3):
                  chunk(1, qg, all_act=True)
              if full and not QSPLIT:
                  for kt in (2, 3):
                      for h in range(NH):
                          score_exp(h, kt, 1, bias_via_pe=BIAS_PE_TAIL)
                  attn_v_block(N // 2, N // 2)
                  oproj(2)
                  oproj(3)
              if full and QSPLIT:
                  attn_v_block(N // 2, N // 4)
                  for h in range(NH):
                      for kt in (2, 3):
                          score_exp(h, kt, q0=3 * N // 4, qn=N // 4)
                  oproj(2)
                  attn_v_block(3 * N // 4, N // 4)
                  oproj(3)
            if _loop_cm is not None:
                _loop_cm.__exit__(None, None, None)

    _strip_self_waits(nc)
    if SEM_CONSOLIDATE:
        _consolidate_sem_incs(nc)
    _fit_sync_limits(nc)
    from concourse.library_overlay import lower_extended_insts
    lower_extended_insts(nc)
    return nc


def _strip_self_waits(nc):
    """Remove same-engine semaphore waits (vacuous: engines execute in
    program order) so instructions fit walrus' per-instruction sync-command
    limits."""
    import concourse.mybir as mybir
    eng_sem = {
        mybir.EngineType.PE: "PE_",
        mybir.EngineType.DVE: "DVE_",
        mybir.EngineType.Activation: "Activation_",
        mybir.EngineType.SP: "SP_",
        mybir.EngineType.Pool: "Pool_",
    }
    for blk in nc.m.functions[0].blocks:
        for i in blk.instructions:
            si = i.sync_info
            if not si or not si.on_wait:
                continue
            eng = getattr(i, "engine", None)
            pref = eng_sem.get(eng)
            if pref is not None:
                kept = [w for w in si.on_wait if not w.ant_name.startswith(pref)]
                if len(kept) != len(si.on_wait):
                    si.on_wait = kept
            # dist-stream DMAs: a PE wait (WAR vs this slot's readers)
            # transitively implies the predecessor DMA completed, making a
            # coexisting cross-lane DMAHW wait redundant.
            if type(i).__name__ == "InstDMACopy" and any(
                "dist_t" in getattr(o, "memref", "") for o in i.outs
            ):
                w = si.on_wait
                if len(w) > 1 and any(x.ant_name.startswith("PE_") for x in w):
                    si.on_wait = [
                        x for x in w if not x.ant_name.startswith("DMAHW")
                    ]


def _consolidate_sem_incs(nc):
    """Drop the per-matmul `sem-inc +1` on all but the last matmul of each
    PSUM-bank group, remapping every wait threshold on that semaphore to the
    new (smaller) cumulative counts.  PE executes its queue in order, so a
    consumer that waited for "32 matmuls done" can equivalently wait for
    "bank-group #1 done"; the EVT_SEM write per matmul is pure overhead.

    Runs break at: a waiting instruction (its wait may transitively depend
    on an earlier group's inc), a different target semaphore, a multi-update
    or non-unit-value update, any non-MM/LDW instruction, and any change of
    output tensor (keeps all existing wait thresholds exactly on run
    boundaries).  Waits are remapped as v -> index of the run containing
    the v'th original inc; mid-run thresholds round up to the run's end,
    which is correct (conservative) because nothing inside a run is waited
    on by anything a run member depends on (first-members carry the run's
    only waits)."""
    CONS_TYPES = {"InstMatmult", "InstLdweights"}

    for fn in nc.m.functions:
        # sems are reset between For_i iterations; each block's waits refer
        # to counts from its own block.  Refuse any sem whose updates span
        # blocks (the wait remap below assumes one producing block).
        sem_block_count = {}
        for blk in fn.blocks:
            seen = set()
            for i in blk.instructions:
                si = i.sync_info
                if not si:
                    continue
                for u in si.on_update:
                    seen.add(u.ant_name)
            for s in seen:
                sem_block_count[s] = sem_block_count.get(s, 0) + 1
        for blk in fn.blocks:
            pe_insts = [
                i for i in blk.instructions
                if getattr(i, "engine", None) is not None
                and getattr(i.engine, "name", "") == "PE"
            ]
            # collect candidate sems updated by PE in this block
            sem_updates = {}
            for i in pe_insts:
                si = i.sync_info
                if not si:
                    continue
                for u in si.on_update:
                    sem_updates.setdefault(u.ant_name, []).append((i, u))
            for sem, upds in sem_updates.items():
                # eligibility: every update on this sem is a lone sem-inc +1
                # on a MM/LDW instruction
                ok = all(
                    type(i).__name__ in CONS_TYPES
                    and len(i.sync_info.on_update) == 1
                    and u.update_mode == "sem-inc"
                    and u.update_value == 1
                    for i, u in upds
                )
                if not ok or len(upds) < 4 or sem_block_count.get(sem, 0) > 1:
                    continue
                # waits on this sem anywhere must be plain immediate geq
                waits = []
                for b2 in fn.blocks:
                    for i in b2.instructions:
                        si = i.sync_info
                        if not si:
                            continue
                        for w in si.on_wait:
                            if w.ant_name == sem:
                                waits.append(w)
                if any(
                    getattr(w, "wait_reg", None) is not None
                    or w.wait_value is None
                    for w in waits
                ):
                    continue
                # form runs over this block's PE instruction order
                run_end_old = []       # old cumulative count at each run end
                survivors = []         # the update-bearing inst ending each run
                cum = 0
                run_len = 0
                run_out = None
                run_last = None

                def flush():
                    nonlocal run_len, run_out, run_last
                    if run_len:
                        run_end_old.append(cum)
                        survivors.append(run_last)
                    run_len = 0
                    run_out = None
                    run_last = None

                for i in pe_insts:
                    si = i.sync_info
                    has_wait = bool(si and si.on_wait)
                    my_upds = [u for u in (si.on_update if si else [])
                               if u.ant_name == sem]
                    if type(i).__name__ not in CONS_TYPES:
                        flush()
                        continue
                    if has_wait and run_len:
                        flush()
                    if not my_upds:
                        if si and si.on_update:
                            flush()   # updates some other sem: break run
                        continue
                    out0 = getattr(i.outs[0], "memref", "") if i.outs else ""
                    if run_len and out0 != run_out:
                        flush()
                    cum += 1
                    run_len += 1
                    run_out = out0
                    run_last = i
                    if SEM_CONS_SCOPE == "bias" and "psC" not in out0:
                        flush()   # consolidate only dist-bias bank groups
                flush()
                if len(survivors) >= len(upds):
                    continue
                # rewrite updates: only survivors keep their +1
                surv_set = {id(i) for i in survivors}
                for i, u in upds:
                    if id(i) not in surv_set:
                        i.sync_info.on_update = [
                            x for x in i.sync_info.on_update
                            if x.ant_name != sem
                        ]
                # remap waits: old threshold v -> first run index r (1-based)
                # with run_end_old[r-1] >= v
                import bisect
                for w in waits:
                    v = w.wait_value
                    if v <= 0:
                        continue
                    r = bisect.bisect_left(run_end_old, v)
                    w.wait_value = min(r + 1, len(run_end_old))


_FITTABLE = {
    "InstMatmult", "InstLdweights", "InstActivation", "InstTensorTensor",
    "InstTensorCopy", "InstTensorScalarPtr", "InstCustomDveAnt",
    "InstMemset", "InstReciprocal", "InstDMACopy", "InstTensorReduce",
    "InstDrain", "InstNoOp", "InstEventSemaphore",
}


def _fit_sync_limits(nc):
    """Walrus' 64B instruction encodings fit 3 sync slots; a wait costs 2,
    an update 1 — so at most ONE wait per instruction.  Hoist excess waits
    onto same-engine NOPs injected just before the instruction — the NX
    sequencer executes the NOP's waits first, which is semantically
    identical."""
    import concourse.mybir as mybir

    for blk in nc.m.functions[0].blocks:
        il = blk.instructions
        out = []
        for inst in il:
            si = inst.sync_info
            if (
                type(inst).__name__ not in _FITTABLE
                or si is None
                or not si.on_wait
            ):
                out.append(inst)
                continue
            waits = list(si.on_wait)
            if len(waits) <= 1:
                out.append(inst)
                continue
            excess, kept = waits[:-1], waits[-1:]
            for j, w in enumerate(excess):
                nop = mybir.InstNoOp(
                    name=f"{inst.name}-hw{j}",
                    engine=inst.engine,
                    ins=[],
                    outs=[],
                    sync_info=mybir.SyncInfo(on_wait=[w], on_update=[]),
                )
                out.append(nop)
            si.on_wait = kept
            out.append(inst)
        il[:] = out


def _get_bass():
    with _lock:
        key = ("nc", DIST_FP8)
        if key not in _cache:
            _cache[key] = _build_bass(dist_fp8=DIST_FP8)
        return _cache[key]


def _prep_core(b, x, dist, mask, bd, ddtype):
    """Build the per-core input map for batch element b."""
    xT = np.ascontiguousarray(x[b].T).astype(BF16)
    d = dist[b].reshape(NQP, 2, NKH, NKW, DD)
    distH = np.ascontiguousarray(d.transpose(2, 1, 4, 0, 3)).reshape(
        NKH, 128, NQP * NKW
    ).astype(ddtype)
    mk = mask[b].astype(np.float32)
    madd = np.where(mk > 0.5, 0.0, -1e9).astype(np.float32)
    madh = np.empty((128, NH * NKT), np.float32)
    for h in range(NH):
        for kt in range(NKT):
            madh[:, h * NKT + kt] = madd[kt * 128:(kt + 1) * 128] + float(bd[h])
    return {
        "distH": distH,
        "xT": xT,
        "madh": madh,
        "mqrow": np.where(mk > 0.5, 1.0, 1e30).astype(
            np.float32).reshape(1, N),
    }


def _cpu_reference(x, dist, mask, Wq, bq, Wk, bk, Wv, bv, Wo, bo, Wd, bd):
    """NumPy fallback for input shapes/bias values the Bass kernel doesn't
    hardcode.  Never taken for the reference setup_inputs()."""
    Bn, Nn, Hn = x.shape
    nh = Wd.shape[1]
    dh = Hn // nh
    sc = float(np.sqrt(dh))

    def heads(t):
        return t.reshape(Bn, Nn, nh, dh).transpose(0, 2, 1, 3)

    q = heads(x @ Wq + bq)
    k = heads(x @ Wk + bk)
    v = heads(x @ Wv + bv)
    scores = np.einsum("bhqd,bhkd->bhqk", q, k) / sc
    scores = scores + (dist @ Wd + bd).transpose(0, 3, 1, 2)
    scores = np.where(mask[:, None, None, :], scores, -1e9)
    scores = scores - scores.max(axis=-1, keepdims=True)
    e = np.exp(scores)
    attn = e / e.sum(axis=-1, keepdims=True)
    attn = attn * mask[:, None, :, None].astype(attn.dtype)
    out = np.einsum("bhqk,bhkd->bhqd", attn, v)
    out = out.transpose(0, 2, 1, 3).reshape(Bn, Nn, Hn)
    out = (out @ Wo + bo) * mask[:, :, None].astype(out.dtype)
    return out.astype(np.float32)


def kernel(x, dist_encoding, mask, Wq, bq, Wk, bk, Wv, bv, Wo, bo, Wd, bd,
           trace=False):
    from concourse.bass_utils import run_bass_kernel_spmd

    x = np.asarray(x, dtype=np.float32)
    dist = np.asarray(dist_encoding, dtype=np.float32)
    mask = np.asarray(mask)
    Wq = np.asarray(Wq, np.float32); Wk = np.asarray(Wk, np.float32)
    Wv = np.asarray(Wv, np.float32); Wo = np.asarray(Wo, np.float32)
    Wd = np.asarray(Wd, np.float32)
    bq = np.asarray(bq, np.float32); bk = np.asarray(bk, np.float32)
    bv = np.asarray(bv, np.float32); bo = np.asarray(bo, np.float32)
    bd = np.asarray(bd, np.float32)
    if (np.any(bq) or np.any(bk) or np.any(bv) or np.any(bo)
            or x.shape != (B, N, H) or dist.shape != (B, N, N, DD)):
        return _cpu_reference(x, dist, mask, Wq, bq, Wk, bk, Wv, bv,
                              Wo, bo, Wd, bd)

    # shared (replicated) weights
    wq_s = np.ascontiguousarray(Wq / SCALE).astype(BF16)
    wk_b = np.ascontiguousarray(Wk).astype(BF16)
    wv_b = np.ascontiguousarray(Wv).astype(BF16)
    wo_b = np.ascontiguousarray(Wo).astype(BF16)
    wdd = np.zeros((128, 16), np.float32)
    wdd[0:64, 0:8] = Wd
    wdd[64:128, 8:16] = Wd
    if DIST_FP8:
        wdd = (wdd * 64.0).astype(FP8)
    else:
        wdd = wdd.astype(BF16)

    from concurrent.futures import ThreadPoolExecutor
    ddtype = FP8 if DIST_FP8 else BF16
    with ThreadPoolExecutor(max_workers=8) as ex:
        percore = list(ex.map(
            lambda b: _prep_core(b, x, dist, mask, bd, ddtype),
            range(B),
        ))
    in_maps = []
    for b in range(B):
        m = dict(percore[b])
        xT_b = m.pop("xT")
        for i, w in enumerate((xT_b, wq_s, wk_b, wv_b, wo_b)):
            m[f"bw{i}"] = np.ascontiguousarray(
                w.reshape(4, 128, H).transpose(1, 0, 2).reshape(128, 4 * H))
        m["wdd"] = wdd
        m["ident"] = np.eye(128, dtype=BF16)
        in_maps.append(m)

    nc = _get_bass()
    kernel.last_in_maps = in_maps
    res = run_bass_kernel_spmd(nc, in_maps, list(range(B)), trace=False)
    out = np.stack([res.results[b]["out"] for b in range(B)]).astype(np.float32)
    if trace:
        kernel.last_exec_time_ns = res.exec_time_ns
        kernel.last_results = res
    return out


def bench_exec_ns(in_maps=None, iters=16, reps2=129, mode='full', dist_eng='sync'):
    """Estimate per-execution HW time: steady-state wall time of the jitted
    SPMD kernel with device-resident inputs, minus bare dispatch overhead."""
    import time
    import jax
    import jax.numpy as jnp
    from jax.sharding import Mesh, PartitionSpec
    from jax.experimental.shard_map import shard_map
    import concourse.bass2jax as b2j
    import concourse.mybir as mybir

    if in_maps is None:
        in_maps = kernel.last_in_maps
    n_cores = len(in_maps)

    nc = _build_bass(mode=mode, dist_eng=dist_eng, loop_reps=1,
                     dist_fp8=DIST_FP8)
    ncR = _build_bass(mode=mode, dist_eng=dist_eng, loop_reps=reps2,
                      dist_fp8=DIST_FP8)
    partition_name = nc.partition_id_tensor.name if nc.partition_id_tensor else None
    in_names, out_names, out_avals, zero_outs = [], [], [], []
    for alloc in nc.m.functions[0].allocations:
        if not isinstance(alloc, mybir.MemoryLocationSet):
            continue
        name = alloc.memorylocations[0].name
        if alloc.kind == "ExternalInput":
            if name != partition_name:
                in_names.append(name)
        elif alloc.kind == "ExternalOutput":
            out_names.append(name)
            shape = tuple(alloc.tensor_shape)
            dtype = mybir.dt.np(alloc.dtype)
            out_avals.append(jax.core.ShapedArray(shape, dtype))
            zero_outs.append(np.zeros(shape, dtype))
    n_params = len(in_names)
    n_outs = len(out_avals)
    all_in_names = list(in_names) + out_names
    if partition_name is not None:
        all_in_names.append(partition_name)

    def _mk_body(nc_):
        def _body(*args):
            operands = list(args)
            if partition_name is not None:
                operands.append(b2j.partition_id_tensor())
            outs = b2j._bass_exec_p.bind(
                *operands,
                out_avals=tuple(out_avals),
                in_names=tuple(all_in_names),
                out_names=tuple(out_names),
                lowering_input_output_aliases=(),
                sim_require_finite=True,
                sim_require_nnan=True,
                nc=nc_,
            )
            return tuple(outs)
        return _body

    devices = jax.devices()[:n_cores]
    mesh = Mesh(np.asarray(devices), ("core",))
    in_specs = (PartitionSpec("core"),) * (n_params + n_outs)
    out_specs = (PartitionSpec("core"),) * n_outs
    def make_fn(nc_):
        return jax.jit(
            shard_map(_mk_body(nc_), mesh=mesh,
                      in_specs=in_specs, out_specs=out_specs, check_rep=False),
            keep_unused=True,
        )

    fn = make_fn(nc)
    from jax.sharding import NamedSharding
    shardng = NamedSharding(mesh, PartitionSpec("core"))
    concat_in = [
        jax.device_put(
            np.concatenate([np.asarray(in_maps[c][in_names[i]])
                            for c in range(n_cores)], axis=0), shardng)
        for i in range(n_params)
    ]
    concat_zeros = [
        jax.device_put(
            np.zeros((n_cores * z.shape[0], *z.shape[1:]), z.dtype), shardng)
        for z in zero_outs
    ]
    fnK = make_fn(ncR)

    args = concat_in + concat_zeros
    jax.block_until_ready(fn(*args))
    jax.block_until_ready(fnK(*args))
    t1s, tKs = [], []
    for _ in range(iters):
        t0 = time.perf_counter()
        jax.block_until_ready(fn(*args))
        t1s.append(time.perf_counter() - t0)
        t0 = time.perf_counter()
        jax.block_until_ready(fnK(*args))
        tKs.append(time.perf_counter() - t0)
    t1s.sort(); tKs.sort()
    k = max(3, iters // 3)
    t1 = sum(t1s[:k]) / k
    tK = sum(tKs[:k]) / k
    per = (tK - t1) / (reps2 - 1)
    return {
        "kernel_wall_ns": t1 * 1e9,
        "kernel_wallK_ns": tK * 1e9,
        "exec_est_ns": per * 1e9,
    }
